# revision 49
# baseline (speedup 1.0000x reference)
"""Trainium2 Bass kernel for MixtureOfSoftmaxes.

Module: RMSNorm -> gate MLP (silu, softmax over K experts) -> big GEMM
x @ expert_w (H=1024 -> K*V=128000), softmax over V per expert, mix with
gate weights, log.

Sharding: tensor-parallel over vocab. Core c owns, for all K=4 experts,
the vocab window [c*4000, (c+1)*4000). The only cross-core quantity is
the per-(token, expert) softmax denominator Z; each core AllReduces its
local partial sums per 128-token block (2 KB each).

v2 design (single fused NEFF):
- The core's weight shard (1024 x 16000 fp8, 16 MB) is fully RESIDENT in
  SBUF, loaded once at kernel start (8 column-group tiles) while RMSNorm
  and the gate MLP run. No weight re-streaming at all.
- GEMM per (token block, 2048-col group): 16 DoubleRow fp8 matmuls into a
  4-bank PSUM tile, then ONE wide Exp activation (psum -> fp8 P in SBUF)
  with accum_out giving the group's row sums.
- P is kept in fp8 (16 KB/partition/block, 3 bufs) so W+P fit in SBUF.
- Per-block AllReduce of [128,4] partial sums; mix+log runs two blocks
  behind the GEMM so the ~20-30us collective latency never backpressures
  the pipeline; the two exposed tail mixes split their products between
  vector and scalar.
- DMA triggers that can wait (Z return, outputs) live on the idle sync
  queue: the tile framework signals dependencies via per-engine ordered
  counters, so a waiting instruction at the head of a busy queue would
  stall unrelated downstream consumers (e.g. matmuls waiting on psum).
- W and x are host-repacked so every DMA reads contiguous per-partition
  blocks (descriptor generation on the sync engine is ~7ns/descriptor).
"""

import sys

sys.path.insert(0, "/opt/trn_rl_repo")

import numpy as np
import ml_dtypes

import concourse.bass as bass
import concourse.bacc as bacc
import concourse.mybir as mybir
import concourse.tile as tile
from concourse.bass_utils import run_bass_kernel_spmd
from concourse.masks import make_identity

AFT = mybir.ActivationFunctionType
F32 = mybir.dt.float32
BF16 = mybir.dt.bfloat16
FP8 = mybir.dt.float8e4
FP8NP = ml_dtypes.float8_e4m3
WSCALE = 16.0

B, S, H, K, V = 2, 512, 1024, 4, 32000
T = B * S              # 1024 tokens
NC = 8                 # cores
VSH = V // NC          # 4000 vocab cols per core per expert
C = K * VSH            # 16000 GEMM cols per core (no padding)
D = H // 2             # 512 gate hidden
EPS_RMS = 1e-5
EPS_LOG = 1e-10
TB = T // 128          # 8 token blocks
HB = H // 128          # 8 contraction blocks
# column groups: per expert [0:2048] and [2048:4000]
GRPS = []
for k in range(K):
    GRPS.append((k * VSH, 2048))
    GRPS.append((k * VSH + 2048, VSH - 2048))
NG = len(GRPS)         # 8 groups
# mix sub-chunks (vector) and Ln chunks (scalar) per vocab window
OCH = [(0, 1000), (1000, 1000), (2000, 1000), (3000, 1000)]
OCW = 1024
LNCH = [(0, 2000), (2000, 2000)]


def build_fused():
    nc = bacc.Bacc("TRN2", target_bir_lowering=False, debug=False, num_devices=NC)
    # x is host-packed [p][t][h]: partition p holds token t*128+p for all
    # 8 blocks — one fully-contiguous DMA (32 KB per partition).
    # x ships as bf16: the normed activations are rounded to bf16 (xb) and
    # fp8 (xT8) downstream anyway, and halving x's bytes pulls the whole
    # input-DMA train (which gates block 0's GEMM) in by ~8us.
    x_d = nc.dram_tensor("x", [128, TB * H], BF16, kind="ExternalInput")
    # w is host-packed per-partition-contiguous: for each column group g,
    # a [128, HB*cw] block where partition p holds rows {hb*128+p} of the
    # group's columns, hb-major. One DMA per group, 16 KB contiguous per
    # partition -> ~128 descriptors instead of 8192.
    w_d = nc.dram_tensor("w", [128, HB * C], FP8, kind="ExternalInput")
    wd_d = nc.dram_tensor("wd", [H, D], FP8, kind="ExternalInput")
    wu_d = nc.dram_tensor("wu", [D, K], BF16, kind="ExternalInput")
    bd_d = nc.dram_tensor("bd", [D, 1], F32, kind="ExternalInput")
    bu_d = nc.dram_tensor("bu", [K, 1], F32, kind="ExternalInput")
    o_d = nc.dram_tensor("o", [TB, 128, VSH], F32, kind="ExternalOutput")

    wd_ap = wd_d.rearrange("(hb p) d -> p hb d", p=128)
    wu_ap = wu_d.rearrange("(db p) k -> p db k", p=128)
    bd_ap = bd_d.rearrange("(db p) o -> p db o", p=128)

    with tile.TileContext(nc) as tc:
        with tc.tile_pool(name="persist", bufs=1) as pers:
            # create ALL persistent tiles up front, BEFORE any scoped pool
            # opens: otherwise later pers tiles land in addresses the scoped
            # pools already occupy, and their DMAs inherit false WAR
            # dependencies on the scoped pools' consumers.
            w_sb = []
            for g, (c0, cw) in enumerate(GRPS):
                w_sb.append(pers.tile([128, HB, cw], FP8, name=f"wg{g}"))
            ident = pers.tile([128, 128], BF16)
            make_identity(nc, ident[:])
            ident32 = pers.tile([4, 4], F32)
            make_identity(nc, ident32[:])
            eps_rms = pers.tile([128, 1], F32)
            nc.gpsimd.memset(eps_rms[:], EPS_RMS)
            eps_log = pers.tile([128, 1], F32)
            nc.gpsimd.memset(eps_log[:], EPS_LOG)
            xT8 = pers.tile([128, HB, T], FP8)     # 8 KB/partition
            ss = pers.tile([128, TB], F32)
            sd = pers.tile([128, TB], F32)
            rinv = pers.tile([128, TB], F32)
            gw = pers.tile([128, TB, K], F32)
            # group sums laid out [t, half, expert] so the per-expert
            # pair-add is a plain elementwise add of the two halves
            schunk = pers.tile([128, TB, 2, K], F32)

            # ---- RMSNorm + transpose to xT8 (h on partitions) ----
            # gate_sb pool opens alongside norm so the gate weight DMAs can
            # issue immediately (fresh addresses, no false WAR deps)
            gsb_ctx = tc.tile_pool(name="gate_sb", bufs=1)
            gsb = gsb_ctx.__enter__()
            with tc.tile_pool(name="norm", bufs=2) as norm_pool, \
                 tc.tile_pool(name="tp_psum", bufs=2, space="PSUM") as tp_psum:
                # sync-queue order: x halves first (norm can start early),
                # then gate weights (small), then the weight-shard groups
                xall = norm_pool.tile([128, TB, H], BF16, bufs=1)
                x_src = x_d[:].rearrange("p (t h) -> p t h", t=TB)
                nc.sync.dma_start(xall[:, : TB // 2, :], x_src[:, : TB // 2, :])
                nc.sync.dma_start(xall[:, TB // 2 :, :], x_src[:, TB // 2 :, :])
                wd_sb = gsb.tile([128, HB, D], FP8)   # 4 KB/partition
                nc.sync.dma_start(wd_sb[:], wd_ap)
                wu_sb = gsb.tile([128, D // 128, K], BF16)
                nc.sync.dma_start(wu_sb[:], wu_ap)
                bd_sb = gsb.tile([128, D // 128, 1], F32)
                nc.sync.dma_start(bd_sb[:], bd_ap)
                bu_sb = gsb.tile([K, 1], F32)
                nc.sync.dma_start(bu_sb[:], bu_d[:])
                # two DMAs per group (hb halves) — more packets in flight
                # across the DMA engines, earlier availability per group.
                # Cap descriptors at 4 KB: smaller descriptors round-robin
                # across all 16 DMA engines (8 KB ones spread poorly and the
                # load crawled at ~250 GB/s instead of ~340).
                off = 0
                for g, (c0, cw) in enumerate(GRPS):
                    half = HB // 2 * cw
                    for j in range(2):
                        nc.sync.dma_start(
                            w_sb[g][:, j * HB // 2 : (j + 1) * HB // 2, :],
                            w_d[:, off + j * half : off + (j + 1) * half].rearrange(
                                "p (h c) -> p h c", h=HB // 2),
                            max_dma_last_dim=4096)
                    off += HB * cw

                for t in range(TB):
                    xt = xall[:, t, :]
                    sq = norm_pool.tile([128, H], F32, tag="sq")
                    nc.scalar.activation(sq[:], xt, AFT.Square, bias=0.0,
                                         scale=1.0, accum_out=ss[:, t : t + 1])
                    nc.scalar.activation(sd[:, t : t + 1], ss[:, t : t + 1],
                                         AFT.Sqrt, bias=eps_rms[:], scale=1.0 / H)
                    nc.vector.reciprocal(rinv[:, t : t + 1], sd[:, t : t + 1])
                    xb = norm_pool.tile([128, H], BF16, tag="xb")
                    nc.scalar.mul(xb[:], xt, rinv[:, t : t + 1])
                    for h in range(HB):
                        tp = tp_psum.tile([128, 128], BF16, tag="tp")
                        nc.tensor.transpose(tp[:], xb[:, h * 128 : (h + 1) * 128], ident[:])
                        nc.vector.tensor_copy(xT8[:, h, t * 128 : (t + 1) * 128], tp[:])

            # ---- gate MLP (fp8 DoubleRow) + on-device softmax -> gw ----
            with tc.tile_pool(name="gate_psum", bufs=1, space="PSUM") as gps:
                gT = gsb.tile([128, D // 128, T], BF16)
                for d in range(D // 128):
                    pg = gps.tile([128, T], F32, tag="pg", name=f"pg{d}", bufs=2)
                    for hs in range(HB // 2):
                        for half in range(2):
                            nc.tensor.matmul(
                                pg[:, half * 512 : (half + 1) * 512],
                                lhsT=wd_sb[:, 2 * hs : 2 * hs + 2, d * 128 : (d + 1) * 128],
                                rhs=xT8[:, 2 * hs : 2 * hs + 2, half * 512 : (half + 1) * 512],
                                start=(hs == 0), stop=(hs == HB // 2 - 1),
                                perf_mode=mybir.MatmulPerfMode.DoubleRow,
                            )
                    lin = gsb.tile([128, T], F32, tag="lin", name=f"lin{d}")
                    nc.vector.tensor_scalar(lin[:], pg[:], 1.0 / WSCALE,
                                            bd_sb[:, d, :],
                                            op0=mybir.AluOpType.mult,
                                            op1=mybir.AluOpType.add)
                    sig = gsb.tile([128, T], F32, tag="sig", name=f"sig{d}")
                    nc.scalar.activation(sig[:], pg[:], AFT.Sigmoid,
                                         bias=bd_sb[:, d, :], scale=1.0 / WSCALE)
                    nc.vector.tensor_mul(gT[:, d, :], lin[:], sig[:])
                pl = gps.tile([K, T], F32, tag="pl", bufs=1)
                for d in range(D // 128):
                    for half in range(2):
                        nc.tensor.matmul(
                            pl[:, half * 512 : (half + 1) * 512],
                            lhsT=wu_sb[:, d, :],
                            rhs=gT[:, d, half * 512 : (half + 1) * 512],
                            start=(d == 0), stop=(d == D // 128 - 1),
                        )
                gl_sb = gsb.tile([K, T], F32)
                nc.scalar.activation(gl_sb[:], pl[:], AFT.Identity,
                                     bias=bu_sb[:], scale=1.0)
                # softmax over K: transpose to t-major then rowwise ops
                glt = gsb.tile([128, TB, K], F32)
                for t in range(TB):
                    gp = gps.tile([128, K], F32, tag="gp", name=f"gp{t}", bufs=2)
                    nc.tensor.transpose(gp[:], gl_sb[:, t * 128 : (t + 1) * 128],
                                        ident32[:])
                    nc.vector.tensor_copy(glt[:, t, :], gp[:])
                negm = gsb.tile([128, TB], F32)
                esum = gsb.tile([128, TB], F32)
                for t in range(TB):
                    nc.vector.tensor_reduce(
                        negm[:, t : t + 1], glt[:, t, :],
                        axis=mybir.AxisListType.X, op=mybir.AluOpType.max,
                        negate=True,
                    )
                    nc.scalar.activation(gw[:, t, :], glt[:, t, :], AFT.Exp,
                                         bias=negm[:, t : t + 1], scale=1.0,
                                         accum_out=esum[:, t : t + 1])
                rsum = gsb.tile([128, TB], F32)
                nc.vector.reciprocal(rsum[:], esum[:])
                for t in range(TB):
                    nc.vector.tensor_scalar_mul(gw[:, t, :], gw[:, t, :],
                                                rsum[:, t : t + 1])
            gsb_ctx.__exit__(None, None, None)

            # ---- main loop: GEMM + exp per block, AR + mix two blocks behind ----
            with tc.tile_pool(name="pfull", bufs=3) as ppool, \
                 tc.tile_pool(name="mix", bufs=2) as mixp, \
                 tc.tile_pool(name="ccdr", bufs=2, space="DRAM") as ccdr, \
                 tc.tile_pool(name="mm_psum", bufs=2, space="PSUM") as mmps:

                pts = {}

                def emit_gemm(t):
                    pt = ppool.tile([128, C], FP8, tag="P", name=f"P{t}")
                    pts[t] = pt
                    for g, (c0, cw) in enumerate(GRPS):
                        PT = mmps.tile([128, 2048], F32, tag="mm",
                                       name=f"mm_{t}_{g}")
                        for hs in range(HB // 2):
                            for ch0 in range(0, cw, 512):
                                chw = min(512, cw - ch0)
                                nc.tensor.matmul(
                                    PT[:, ch0 : ch0 + chw],
                                    lhsT=xT8[:, 2 * hs : 2 * hs + 2, t * 128 : (t + 1) * 128],
                                    rhs=w_sb[g][:, 2 * hs : 2 * hs + 2, ch0 : ch0 + chw],
                                    start=(hs == 0), stop=(hs == HB // 2 - 1),
                                    perf_mode=mybir.MatmulPerfMode.DoubleRow,
                                )
                        nc.scalar.activation(pt[:, c0 : c0 + cw], PT[:, :cw],
                                             AFT.Exp, bias=0.0, scale=1.0 / WSCALE)
                        # row sums on vector (reads the fp8 P just written):
                        # keeps READ_ACCUMULATOR off the scalar queue, whose
                        # ordered counter paces the matmuls' psum reuse
                        nc.vector.tensor_reduce(
                            schunk[:, t, g % 2, g // 2 : g // 2 + 1],
                            pt[:, c0 : c0 + cw],
                            axis=mybir.AxisListType.X, op=mybir.AluOpType.add)

                def emit_reduce(t):
                    # pair-add group sums -> [128, K]; AllReduce (2 KB)
                    s4 = mixp.tile([128, K], F32, tag="s4", name=f"s4_{t}")
                    nc.gpsimd.tensor_add(s4[:], schunk[:, t, 0, :],
                                         schunk[:, t, 1, :])
                    bi = ccdr.tile([128, K], F32, tag="bi", name=f"bi{t}")
                    bo = ccdr.tile([128, K], F32, tag="bo", name=f"bo{t}")
                    nc.gpsimd.dma_start(bi[:], s4[:])
                    nc.gpsimd.collective_compute(
                        "AllReduce", mybir.AluOpType.add,
                        replica_groups=[list(range(NC))],
                        ins=[bi[:]], outs=[bo[:]],
                    )
                    return bo

                def emit_mix(t, bo, assist=False):
                    # z/o DMA triggers live on the sync queue (idle during
                    # the main loop) so their waits never head-of-line block
                    # the gpsimd (CC) or scalar (Exp/Ln) queues.
                    z4 = mixp.tile([128, K], F32, tag="z4", name=f"z4_{t}")
                    nc.sync.dma_start(z4[:], bo[:])
                    a4 = mixp.tile([128, K], F32, tag="a4", name=f"a4_{t}")
                    nc.vector.reciprocal(a4[:], z4[:])
                    nc.vector.tensor_mul(a4[:], a4[:], gw[:, t, :])
                    pt = pts.pop(t)
                    # one full-width red per block; mix sub-chunks write
                    # disjoint slices (region-level deps let them pipeline),
                    # then two wide Lns (fewer act-table switches)
                    red = mixp.tile([128, VSH], BF16, tag="red",
                                    name=f"red{t}", bufs=1)
                    for (c0, cw) in OCH:
                        rc = red[:, c0 : c0 + cw]
                        pk = [pt[:, k * VSH + c0 : k * VSH + c0 + cw]
                              for k in range(K)]
                        mk = mixp.tile([128, OCW], BF16, tag="mk",
                                       name=f"mk{t}_{c0}")
                        if assist:
                            # the exposed-tail blocks: scalar does two of the
                            # four products so vector and scalar split the
                            # serial mix chain roughly in half
                            mks = mixp.tile([128, OCW], BF16, tag="mks",
                                            name=f"mks{t}_{c0}")
                            nc.scalar.mul(mks[:, :cw], pk[1], a4[:, 1:2])
                            nc.vector.tensor_scalar_mul(rc, pk[0], a4[:, 0:1])
                            nc.vector.tensor_scalar_mul(mk[:, :cw], pk[2],
                                                        a4[:, 2:3])
                            nc.vector.tensor_add(rc, rc, mk[:, :cw])
                            nc.vector.tensor_add(rc, rc, mks[:, :cw])
                            nc.scalar.mul(mk[:, :cw], pk[3], a4[:, 3:4])
                            nc.vector.tensor_add(rc, rc, mk[:, :cw])
                        else:
                            for k in range(K):
                                if k == 0:
                                    nc.vector.tensor_scalar_mul(rc, pk[0],
                                                                a4[:, 0:1])
                                else:
                                    nc.vector.tensor_scalar_mul(
                                        mk[:, :cw], pk[k], a4[:, k : k + 1])
                                    nc.vector.tensor_add(rc, rc, mk[:, :cw])
                    if not assist:
                        # gate BOTH Lns on the end of the mix (a [128,1] eps
                        # bias derived from red's last column, written by the
                        # in-order vector queue after the final mix chunk):
                        # they become ready together, so the scheduler runs
                        # them adjacently -> one Exp<->Ln act-table switch
                        # pair per block instead of two or more.
                        eps4 = mixp.tile([128, 1], F32, tag="eps4",
                                         name=f"eps4_{t}")
                        nc.vector.tensor_scalar(eps4[:], red[:, VSH - 1 : VSH],
                                                0.0, EPS_LOG,
                                                op0=mybir.AluOpType.mult,
                                                op1=mybir.AluOpType.add)
                        lbias = eps4[:]
                    else:
                        lbias = eps_log[:]
                    for (c0, cw) in LNCH:
                        ot = mixp.tile([128, 2000], F32, tag="ot",
                                       name=f"ot{t}_{c0}", bufs=1)
                        nc.scalar.activation(ot[:, :cw], red[:, c0 : c0 + cw],
                                             AFT.Ln, bias=lbias, scale=1.0)
                        nc.sync.dma_start(o_d[t, :, c0 : c0 + cw], ot[:, :cw])

                # mix(t-2) is emitted after gemm(t): two full blocks of GEMM
                # (~68us) separate a block's AllReduce issue from the point
                # its result is consumed, so AR latency/jitter (~20-30us)
                # never backpressures the GEMM pipeline.
                bos = {}
                for t in range(TB):
                    emit_gemm(t)
                    if t > 1:
                        emit_mix(t - 2, bos.pop(t - 2))
                    bos[t] = emit_reduce(t)
                emit_mix(TB - 2, bos.pop(TB - 2), assist=True)
                emit_mix(TB - 1, bos.pop(TB - 1), assist=True)
    nc.compile()
    return nc


_CACHE = {}


def _get_kernels():
    if "f" not in _CACHE:
        _CACHE["f"] = build_fused()
    return _CACHE["f"]


def kernel(hidden_states, rms_scale, gate_down_w, gate_down_b, gate_up_w,
           gate_up_b, expert_w, trace=False):
    nc_f = _get_kernels()
    core_ids = list(range(NC))

    x = np.asarray(hidden_states, dtype=np.float32).reshape(TB, 128, H)
    # pack [p][t][h]: partition p holds token t*128+p for all blocks
    xp = np.ascontiguousarray(
        x.transpose(1, 0, 2).reshape(128, TB * H)).astype(ml_dtypes.bfloat16)
    scale = np.asarray(rms_scale, dtype=np.float32)
    # fold rms_scale into every weight that consumes the normed activations
    wd = (np.asarray(gate_down_w, dtype=np.float32) * scale[:, None]
          * WSCALE).astype(FP8NP)
    wu = np.asarray(gate_up_w, dtype=np.float32).astype(ml_dtypes.bfloat16)
    bd = np.ascontiguousarray(np.asarray(gate_down_b, dtype=np.float32).reshape(D, 1))
    bu = np.ascontiguousarray(np.asarray(gate_up_b, dtype=np.float32).reshape(K, 1))
    we = np.asarray(expert_w, dtype=np.float32) * scale[:, None]
    we8 = (we * WSCALE).astype(FP8NP).reshape(HB, 128, K, V)

    in_maps = []
    for c in range(NC):
        # per column group g: [128, HB*cw] block, partition-major then
        # hb-major then columns (matches the SBUF tile layout exactly)
        blocks = []
        for (c0, cw) in GRPS:
            k, j0 = c0 // VSH, c0 % VSH
            blk = we8[:, :, k, c * VSH + j0 : c * VSH + j0 + cw]
            blocks.append(blk.transpose(1, 0, 2).reshape(128, HB * cw))
        wsh = np.ascontiguousarray(np.concatenate(blocks, axis=1))
        in_maps.append({"x": xp, "w": wsh, "wd": wd, "wu": wu, "bd": bd, "bu": bu})

    res = run_bass_kernel_spmd(nc_f, in_maps, core_ids, trace=trace)

    out = np.empty((T, V), dtype=np.float32)
    for c in range(NC):
        out[:, c * VSH : (c + 1) * VSH] = res.results[c]["o"].reshape(T, VSH)
    out = out.reshape(B, S, V)
    if trace:
        return out, (res, res)
    return out


# revision 50
# speedup vs baseline: 1.1407x; 1.1407x over previous
"""Trainium2 Bass kernel for MixtureOfSoftmaxes.

Module: RMSNorm -> gate MLP (silu, softmax over K experts) -> big GEMM
x @ expert_w (H=1024 -> K*V=128000), softmax over V per expert, mix with
gate weights, log.

Sharding: tensor-parallel over vocab. Core c owns, for all K=4 experts,
the vocab window [c*4000, (c+1)*4000). The only cross-core quantity is
the per-(token, expert) softmax denominator Z; each core AllReduces its
local partial sums per 128-token block (2 KB each).

v2 design (single fused NEFF):
- The core's weight shard (1024 x 16000 fp8, 16 MB) is fully RESIDENT in
  SBUF, loaded once at kernel start (8 column-group tiles) while RMSNorm
  and the gate MLP run. No weight re-streaming at all.
- GEMM per (token block, 2048-col group): 16 DoubleRow fp8 matmuls into a
  4-bank PSUM tile, then ONE wide Exp activation (psum -> fp8 P in SBUF)
  with accum_out giving the group's row sums.
- P is kept in fp8 (16 KB/partition/block, 3 bufs) so W+P fit in SBUF.
- Per-block AllReduce of [128,4] partial sums; mix+log runs two blocks
  behind the GEMM so the ~20-30us collective latency never backpressures
  the pipeline; the two exposed tail mixes split their products between
  vector and scalar.
- DMA triggers that can wait (Z return, outputs) live on the idle sync
  queue: the tile framework signals dependencies via per-engine ordered
  counters, so a waiting instruction at the head of a busy queue would
  stall unrelated downstream consumers (e.g. matmuls waiting on psum).
- W and x are host-repacked so every DMA reads contiguous per-partition
  blocks (descriptor generation on the sync engine is ~7ns/descriptor).
"""

import sys

sys.path.insert(0, "/opt/trn_rl_repo")

import numpy as np
import ml_dtypes

import concourse.bass as bass
import concourse.bacc as bacc
import concourse.mybir as mybir
import concourse.tile as tile
from concourse.bass_utils import run_bass_kernel_spmd
from concourse.masks import make_identity

AFT = mybir.ActivationFunctionType
F32 = mybir.dt.float32
BF16 = mybir.dt.bfloat16
FP8 = mybir.dt.float8e4
FP8NP = ml_dtypes.float8_e4m3
WSCALE = 16.0

B, S, H, K, V = 2, 512, 1024, 4, 32000
T = B * S              # 1024 tokens
NC = 8                 # cores
VSH = V // NC          # 4000 vocab cols per core per expert
C = K * VSH            # 16000 GEMM cols per core (no padding)
D = H // 2             # 512 gate hidden
EPS_RMS = 1e-5
EPS_LOG = 1e-10
TB = T // 128          # 8 token blocks
HB = H // 128          # 8 contraction blocks
# column groups: per expert [0:2048] and [2048:4000]
GRPS = []
for k in range(K):
    GRPS.append((k * VSH, 2048))
    GRPS.append((k * VSH + 2048, VSH - 2048))
NG = len(GRPS)         # 8 groups
# mix sub-chunks (vector) and Ln chunks (scalar) per vocab window
OCH = [(0, 1000), (1000, 1000), (2000, 1000), (3000, 1000)]
OCW = 1024
LNCH = [(0, 2000), (2000, 2000)]


def build_fused():
    nc = bacc.Bacc("TRN2", target_bir_lowering=False, debug=False, num_devices=NC)
    # x is host-packed [p][t][h]: partition p holds token t*128+p for all
    # 8 blocks — one fully-contiguous DMA (32 KB per partition).
    # x ships as bf16: the normed activations are rounded to bf16 (xb) and
    # fp8 (xT8) downstream anyway, and halving x's bytes pulls the whole
    # input-DMA train (which gates block 0's GEMM) in by ~8us.
    x_d = nc.dram_tensor("x", [128, TB * H], BF16, kind="ExternalInput")
    # w is host-packed per-partition-contiguous: for each column group g,
    # a [128, HB*cw] block where partition p holds rows {hb*128+p} of the
    # group's columns, hb-major. One DMA per group, 16 KB contiguous per
    # partition -> ~128 descriptors instead of 8192.
    w_d = nc.dram_tensor("w", [128, HB * C], FP8, kind="ExternalInput")
    wd_d = nc.dram_tensor("wd", [H, D], FP8, kind="ExternalInput")
    wu_d = nc.dram_tensor("wu", [D, K], BF16, kind="ExternalInput")
    bd_d = nc.dram_tensor("bd", [D, 1], F32, kind="ExternalInput")
    bu_d = nc.dram_tensor("bu", [K, 1], F32, kind="ExternalInput")
    o_d = nc.dram_tensor("o", [TB, 128, VSH], F32, kind="ExternalOutput")

    wd_ap = wd_d.rearrange("(hb p) d -> p hb d", p=128)
    wu_ap = wu_d.rearrange("(db p) k -> p db k", p=128)
    bd_ap = bd_d.rearrange("(db p) o -> p db o", p=128)

    with tile.TileContext(nc) as tc:
        with tc.tile_pool(name="persist", bufs=1) as pers:
            # create ALL persistent tiles up front, BEFORE any scoped pool
            # opens: otherwise later pers tiles land in addresses the scoped
            # pools already occupy, and their DMAs inherit false WAR
            # dependencies on the scoped pools' consumers.
            w_sb = []
            for g, (c0, cw) in enumerate(GRPS):
                w_sb.append(pers.tile([128, HB, cw], FP8, name=f"wg{g}"))
            ident = pers.tile([128, 128], BF16)
            make_identity(nc, ident[:])
            ident32 = pers.tile([4, 4], F32)
            make_identity(nc, ident32[:])
            eps_rms = pers.tile([128, 1], F32)
            nc.gpsimd.memset(eps_rms[:], EPS_RMS)
            eps_log = pers.tile([128, 1], F32)
            nc.gpsimd.memset(eps_log[:], EPS_LOG)
            xT8 = pers.tile([128, HB, T], FP8)     # 8 KB/partition
            ss = pers.tile([128, TB], F32)
            sd = pers.tile([128, TB], F32)
            rinv = pers.tile([128, TB], F32)
            gw = pers.tile([128, TB, K], F32)
            # group sums laid out [t, half, expert] so the per-expert
            # pair-add is a plain elementwise add of the two halves
            schunk = pers.tile([128, TB, 2, K], F32)

            # ---- RMSNorm + transpose to xT8 (h on partitions) ----
            # gate_sb pool opens alongside norm so the gate weight DMAs can
            # issue immediately (fresh addresses, no false WAR deps)
            gsb_ctx = tc.tile_pool(name="gate_sb", bufs=1)
            gsb = gsb_ctx.__enter__()
            with tc.tile_pool(name="norm", bufs=2) as norm_pool, \
                 tc.tile_pool(name="tp_psum", bufs=2, space="PSUM") as tp_psum:
                # sync-queue order: x halves first (norm can start early),
                # then gate weights (small), then the weight-shard groups
                xall = norm_pool.tile([128, TB, H], BF16, bufs=1)
                x_src = x_d[:].rearrange("p (t h) -> p t h", t=TB)
                nc.sync.dma_start(xall[:, : TB // 2, :], x_src[:, : TB // 2, :])
                nc.sync.dma_start(xall[:, TB // 2 :, :], x_src[:, TB // 2 :, :])
                wd_sb = gsb.tile([128, HB, D], FP8)   # 4 KB/partition
                nc.sync.dma_start(wd_sb[:], wd_ap)
                wu_sb = gsb.tile([128, D // 128, K], BF16)
                nc.sync.dma_start(wu_sb[:], wu_ap)
                bd_sb = gsb.tile([128, D // 128, 1], F32)
                nc.sync.dma_start(bd_sb[:], bd_ap)
                bu_sb = gsb.tile([K, 1], F32)
                nc.sync.dma_start(bu_sb[:], bu_d[:])
                # two DMAs per group (hb halves) — more packets in flight
                # across the DMA engines, earlier availability per group.
                # Cap descriptors at 4 KB: smaller descriptors round-robin
                # across all 16 DMA engines (8 KB ones spread poorly and the
                # load crawled at ~250 GB/s instead of ~340).
                off = 0
                for g, (c0, cw) in enumerate(GRPS):
                    half = HB // 2 * cw
                    for j in range(2):
                        nc.sync.dma_start(
                            w_sb[g][:, j * HB // 2 : (j + 1) * HB // 2, :],
                            w_d[:, off + j * half : off + (j + 1) * half].rearrange(
                                "p (h c) -> p h c", h=HB // 2),
                            max_dma_last_dim=4096)
                    off += HB * cw

                for t in range(TB):
                    xt = xall[:, t, :]
                    sq = norm_pool.tile([128, H], F32, tag="sq")
                    nc.scalar.activation(sq[:], xt, AFT.Square, bias=0.0,
                                         scale=1.0, accum_out=ss[:, t : t + 1])
                    nc.scalar.activation(sd[:, t : t + 1], ss[:, t : t + 1],
                                         AFT.Sqrt, bias=eps_rms[:], scale=1.0 / H)
                    nc.vector.reciprocal(rinv[:, t : t + 1], sd[:, t : t + 1])
                    xb = norm_pool.tile([128, H], BF16, tag="xb")
                    nc.scalar.mul(xb[:], xt, rinv[:, t : t + 1])
                    for h in range(HB):
                        tp = tp_psum.tile([128, 128], BF16, tag="tp")
                        nc.tensor.transpose(tp[:], xb[:, h * 128 : (h + 1) * 128], ident[:])
                        nc.vector.tensor_copy(xT8[:, h, t * 128 : (t + 1) * 128], tp[:])

            # ---- gate MLP (fp8 DoubleRow) + on-device softmax -> gw ----
            with tc.tile_pool(name="gate_psum", bufs=1, space="PSUM") as gps:
                gT = gsb.tile([128, D // 128, T], BF16)
                for d in range(D // 128):
                    pg = gps.tile([128, T], F32, tag="pg", name=f"pg{d}", bufs=2)
                    for hs in range(HB // 2):
                        for half in range(2):
                            nc.tensor.matmul(
                                pg[:, half * 512 : (half + 1) * 512],
                                lhsT=wd_sb[:, 2 * hs : 2 * hs + 2, d * 128 : (d + 1) * 128],
                                rhs=xT8[:, 2 * hs : 2 * hs + 2, half * 512 : (half + 1) * 512],
                                start=(hs == 0), stop=(hs == HB // 2 - 1),
                                perf_mode=mybir.MatmulPerfMode.DoubleRow,
                            )
                    lin = gsb.tile([128, T], F32, tag="lin", name=f"lin{d}")
                    nc.vector.tensor_scalar(lin[:], pg[:], 1.0 / WSCALE,
                                            bd_sb[:, d, :],
                                            op0=mybir.AluOpType.mult,
                                            op1=mybir.AluOpType.add)
                    sig = gsb.tile([128, T], F32, tag="sig", name=f"sig{d}")
                    nc.scalar.activation(sig[:], pg[:], AFT.Sigmoid,
                                         bias=bd_sb[:, d, :], scale=1.0 / WSCALE)
                    nc.vector.tensor_mul(gT[:, d, :], lin[:], sig[:])
                pl = gps.tile([K, T], F32, tag="pl", bufs=1)
                for d in range(D // 128):
                    for half in range(2):
                        nc.tensor.matmul(
                            pl[:, half * 512 : (half + 1) * 512],
                            lhsT=wu_sb[:, d, :],
                            rhs=gT[:, d, half * 512 : (half + 1) * 512],
                            start=(d == 0), stop=(d == D // 128 - 1),
                        )
                gl_sb = gsb.tile([K, T], F32)
                nc.scalar.activation(gl_sb[:], pl[:], AFT.Identity,
                                     bias=bu_sb[:], scale=1.0)
                # softmax over K: transpose to t-major then rowwise ops
                glt = gsb.tile([128, TB, K], F32)
                for t in range(TB):
                    gp = gps.tile([128, K], F32, tag="gp", name=f"gp{t}", bufs=2)
                    nc.tensor.transpose(gp[:], gl_sb[:, t * 128 : (t + 1) * 128],
                                        ident32[:])
                    nc.vector.tensor_copy(glt[:, t, :], gp[:])
                negm = gsb.tile([128, TB], F32)
                esum = gsb.tile([128, TB], F32)
                for t in range(TB):
                    nc.vector.tensor_reduce(
                        negm[:, t : t + 1], glt[:, t, :],
                        axis=mybir.AxisListType.X, op=mybir.AluOpType.max,
                        negate=True,
                    )
                    nc.scalar.activation(gw[:, t, :], glt[:, t, :], AFT.Exp,
                                         bias=negm[:, t : t + 1], scale=1.0,
                                         accum_out=esum[:, t : t + 1])
                rsum = gsb.tile([128, TB], F32)
                nc.vector.reciprocal(rsum[:], esum[:])
                for t in range(TB):
                    nc.vector.tensor_scalar_mul(gw[:, t, :], gw[:, t, :],
                                                rsum[:, t : t + 1])
            gsb_ctx.__exit__(None, None, None)

            # ---- main loop: GEMM + exp per block, AR + mix two blocks behind ----
            with tc.tile_pool(name="pfull", bufs=3) as ppool, \
                 tc.tile_pool(name="mix", bufs=2) as mixp, \
                 tc.tile_pool(name="ccdr", bufs=2, space="DRAM") as ccdr, \
                 tc.tile_pool(name="mm_psum", bufs=2, space="PSUM") as mmps:

                pts = {}

                def emit_gemm(t):
                    pt = ppool.tile([128, C], FP8, tag="P", name=f"P{t}")
                    pts[t] = pt
                    for g, (c0, cw) in enumerate(GRPS):
                        PT = mmps.tile([128, 2048], F32, tag="mm",
                                       name=f"mm_{t}_{g}")
                        for hs in range(HB // 2):
                            for ch0 in range(0, cw, 512):
                                chw = min(512, cw - ch0)
                                nc.tensor.matmul(
                                    PT[:, ch0 : ch0 + chw],
                                    lhsT=xT8[:, 2 * hs : 2 * hs + 2, t * 128 : (t + 1) * 128],
                                    rhs=w_sb[g][:, 2 * hs : 2 * hs + 2, ch0 : ch0 + chw],
                                    start=(hs == 0), stop=(hs == HB // 2 - 1),
                                    perf_mode=mybir.MatmulPerfMode.DoubleRow,
                                )
                        nc.scalar.activation(pt[:, c0 : c0 + cw], PT[:, :cw],
                                             AFT.Exp, bias=0.0, scale=1.0 / WSCALE,
                                             accum_out=schunk[:, t, g % 2,
                                                              g // 2 : g // 2 + 1])

                def emit_reduce(t):
                    # pair-add group sums -> [128, K]; AllReduce (2 KB)
                    s4 = mixp.tile([128, K], F32, tag="s4", name=f"s4_{t}")
                    nc.gpsimd.tensor_add(s4[:], schunk[:, t, 0, :],
                                         schunk[:, t, 1, :])
                    bi = ccdr.tile([128, K], F32, tag="bi", name=f"bi{t}")
                    bo = ccdr.tile([128, K], F32, tag="bo", name=f"bo{t}")
                    nc.gpsimd.dma_start(bi[:], s4[:])
                    nc.gpsimd.collective_compute(
                        "AllReduce", mybir.AluOpType.add,
                        replica_groups=[list(range(NC))],
                        ins=[bi[:]], outs=[bo[:]],
                    )
                    return bo

                def emit_mix(t, bo, assist=False):
                    # z/o DMA triggers live on the sync queue (idle during
                    # the main loop) so their waits never head-of-line block
                    # the gpsimd (CC) or scalar (Exp/Ln) queues.
                    z4 = mixp.tile([128, K], F32, tag="z4", name=f"z4_{t}")
                    nc.sync.dma_start(z4[:], bo[:])
                    a4 = mixp.tile([128, K], F32, tag="a4", name=f"a4_{t}")
                    nc.vector.reciprocal(a4[:], z4[:])
                    nc.vector.tensor_mul(a4[:], a4[:], gw[:, t, :])
                    pt = pts.pop(t)
                    # one full-width red per block; mix sub-chunks write
                    # disjoint slices (region-level deps let them pipeline),
                    # then two wide Lns (fewer act-table switches)
                    red = mixp.tile([128, VSH], BF16, tag="red",
                                    name=f"red{t}", bufs=1)
                    for (c0, cw) in OCH:
                        rc = red[:, c0 : c0 + cw]
                        pk = [pt[:, k * VSH + c0 : k * VSH + c0 + cw]
                              for k in range(K)]
                        mk = mixp.tile([128, OCW], BF16, tag="mk",
                                       name=f"mk{t}_{c0}")
                        if assist:
                            # the exposed-tail blocks: scalar does two of the
                            # four products so vector and scalar split the
                            # serial mix chain roughly in half
                            mks = mixp.tile([128, OCW], BF16, tag="mks",
                                            name=f"mks{t}_{c0}")
                            nc.scalar.mul(mks[:, :cw], pk[1], a4[:, 1:2])
                            nc.vector.tensor_scalar_mul(rc, pk[0], a4[:, 0:1])
                            nc.vector.tensor_scalar_mul(mk[:, :cw], pk[2],
                                                        a4[:, 2:3])
                            nc.vector.tensor_add(rc, rc, mk[:, :cw])
                            nc.vector.tensor_add(rc, rc, mks[:, :cw])
                            nc.scalar.mul(mk[:, :cw], pk[3], a4[:, 3:4])
                            nc.vector.tensor_add(rc, rc, mk[:, :cw])
                        else:
                            for k in range(K):
                                if k == 0:
                                    nc.vector.tensor_scalar_mul(rc, pk[0],
                                                                a4[:, 0:1])
                                else:
                                    nc.vector.tensor_scalar_mul(
                                        mk[:, :cw], pk[k], a4[:, k : k + 1])
                                    nc.vector.tensor_add(rc, rc, mk[:, :cw])
                    if not assist:
                        # gate BOTH Lns on the end of the mix (a [128,1] eps
                        # bias derived from red's last column, written by the
                        # in-order vector queue after the final mix chunk):
                        # they become ready together, so the scheduler runs
                        # them adjacently -> one Exp<->Ln act-table switch
                        # pair per block instead of two or more.
                        eps4 = mixp.tile([128, 1], F32, tag="eps4",
                                         name=f"eps4_{t}")
                        nc.vector.tensor_scalar(eps4[:], red[:, VSH - 1 : VSH],
                                                0.0, EPS_LOG,
                                                op0=mybir.AluOpType.mult,
                                                op1=mybir.AluOpType.add)
                        lbias = eps4[:]
                    else:
                        lbias = eps_log[:]
                    for (c0, cw) in LNCH:
                        ot = mixp.tile([128, 2000], F32, tag="ot",
                                       name=f"ot{t}_{c0}", bufs=1)
                        nc.scalar.activation(ot[:, :cw], red[:, c0 : c0 + cw],
                                             AFT.Ln, bias=lbias, scale=1.0)
                        nc.sync.dma_start(o_d[t, :, c0 : c0 + cw], ot[:, :cw])

                # mix(t-2) is emitted after gemm(t): two full blocks of GEMM
                # (~68us) separate a block's AllReduce issue from the point
                # its result is consumed, so AR latency/jitter (~20-30us)
                # never backpressures the GEMM pipeline.
                bos = {}
                for t in range(TB):
                    emit_gemm(t)
                    if t > 1:
                        emit_mix(t - 2, bos.pop(t - 2))
                    bos[t] = emit_reduce(t)
                emit_mix(TB - 2, bos.pop(TB - 2), assist=True)
                emit_mix(TB - 1, bos.pop(TB - 1), assist=True)
    nc.compile()
    return nc


_CACHE = {}


def _get_kernels():
    if "f" not in _CACHE:
        _CACHE["f"] = build_fused()
    return _CACHE["f"]


def kernel(hidden_states, rms_scale, gate_down_w, gate_down_b, gate_up_w,
           gate_up_b, expert_w, trace=False):
    nc_f = _get_kernels()
    core_ids = list(range(NC))

    x = np.asarray(hidden_states, dtype=np.float32).reshape(TB, 128, H)
    # pack [p][t][h]: partition p holds token t*128+p for all blocks
    xp = np.ascontiguousarray(
        x.transpose(1, 0, 2).reshape(128, TB * H)).astype(ml_dtypes.bfloat16)
    scale = np.asarray(rms_scale, dtype=np.float32)
    # fold rms_scale into every weight that consumes the normed activations
    wd = (np.asarray(gate_down_w, dtype=np.float32) * scale[:, None]
          * WSCALE).astype(FP8NP)
    wu = np.asarray(gate_up_w, dtype=np.float32).astype(ml_dtypes.bfloat16)
    bd = np.ascontiguousarray(np.asarray(gate_down_b, dtype=np.float32).reshape(D, 1))
    bu = np.ascontiguousarray(np.asarray(gate_up_b, dtype=np.float32).reshape(K, 1))
    we = np.asarray(expert_w, dtype=np.float32) * scale[:, None]
    we8 = (we * WSCALE).astype(FP8NP).reshape(HB, 128, K, V)

    in_maps = []
    for c in range(NC):
        # per column group g: [128, HB*cw] block, partition-major then
        # hb-major then columns (matches the SBUF tile layout exactly)
        blocks = []
        for (c0, cw) in GRPS:
            k, j0 = c0 // VSH, c0 % VSH
            blk = we8[:, :, k, c * VSH + j0 : c * VSH + j0 + cw]
            blocks.append(blk.transpose(1, 0, 2).reshape(128, HB * cw))
        wsh = np.ascontiguousarray(np.concatenate(blocks, axis=1))
        in_maps.append({"x": xp, "w": wsh, "wd": wd, "wu": wu, "bd": bd, "bu": bu})

    res = run_bass_kernel_spmd(nc_f, in_maps, core_ids, trace=trace)

    out = np.empty((T, V), dtype=np.float32)
    for c in range(NC):
        out[:, c * VSH : (c + 1) * VSH] = res.results[c]["o"].reshape(T, VSH)
    out = out.reshape(B, S, V)
    if trace:
        return out, (res, res)
    return out
